# revision 2
# baseline (speedup 1.0000x reference)
"""Grouped-query attention + output projection on 8 trn2 NeuronCores.

Sharding: KV group g (and its 4 query heads) -> core g.  Each core computes
its group's attention entirely locally in a transposed layout (scores^T =
[k, q]) so no on-device transposes are needed anywhere:

  mm1:   scoresT[k, q] = kT_tile.T @ qT          (contraction over D=128)
  exp:   ACT Exp with fused 1/sqrt(D) scale, PSUM -> SBUF (fp32r)
  denom: ones[128,1].T @ expT  -> [1, q]         (accumulated over k tiles)
  mm2:   outT[d, q]  = v_tile.T @ expT           (accumulated over k tiles)
  norm:  outT * (ones x 1/denom)                 (broadcast via K=1 matmul)

The per-core attention outputs (concatT slices [512, 2048]) are AllGathered,
then each core computes a 512-column slice of the output projection:
  out[:, mslice] = concat.T @ w_out[mslice, :].T
Host concatenates the 8 column slices.  All matmuls run as float32r
(full-rate fp32 with reduced-precision multiply); walrus requires every
matmul operand to be produced with dtype float32r, so operand tiles and the
DRAM tensors feeding them are declared float32r (bit-identical to fp32).

Host-side prep transposes Q/K and the w_out slice so every operand lands in
SBUF in the exact layout the tensor engine wants.
"""

import sys

import numpy as np

S = 2048
H = 32
G = 8
D = 128
HPG = H // G          # 4 heads per group/core
MODEL = H * D         # 4096
NCORES = 8
MS = MODEL // NCORES  # 512 output columns per core
JS = HPG * D          # 512 concat rows per core
QC = 512              # q-chunk (matmul free dim)
NQC = S // QC         # 4
NKT = S // 128        # 16 k tiles
NJT = MODEL // 128    # 32 j tiles (proj contraction)
SC = 512              # proj s superchunk
NSC = S // SC         # 4

_CACHE = {}


def _build_bass():
    if "/opt/trn_rl_repo" not in sys.path:
        sys.path.insert(0, "/opt/trn_rl_repo")
    import concourse.bacc as bacc
    import concourse.mybir as mybir
    import concourse.tile as tile

    f32 = mybir.dt.float32
    f32r = mybir.dt.float32r
    EXP = mybir.ActivationFunctionType.Exp
    COPY = mybir.ActivationFunctionType.Copy
    scale = float(D) ** -0.5

    nc = bacc.Bacc(None, num_devices=NCORES)
    qT = nc.dram_tensor("qT", [HPG, D, S], f32r, kind="ExternalInput")
    kT = nc.dram_tensor("kT", [D, S], f32r, kind="ExternalInput")
    v = nc.dram_tensor("v", [S, D], f32r, kind="ExternalInput")
    wT = nc.dram_tensor("wT", [MODEL, MS], f32r, kind="ExternalInput")
    ones_d = nc.dram_tensor("ones", [128, 128], f32r, kind="ExternalInput")
    out = nc.dram_tensor("out", [S, MS], f32, kind="ExternalOutput")

    lp = nc.allow_low_precision("fp32r matmul operands")
    lp.__enter__()
    with tile.TileContext(nc) as tc:
        with (
            tc.tile_pool(name="const", bufs=1) as constp,
            tc.tile_pool(name="kv", bufs=1) as kvp,
            tc.tile_pool(name="w", bufs=1) as wp,
            tc.tile_pool(name="qt", bufs=3) as qtp,
            tc.tile_pool(name="expt", bufs=18) as expp,
            tc.tile_pool(name="cc", bufs=3) as ccp,
            tc.tile_pool(name="misc", bufs=4) as miscp,
            tc.tile_pool(name="proj_in", bufs=10) as pip,
            tc.tile_pool(name="out_sb", bufs=3) as outp,
            tc.tile_pool(name="ps_s", bufs=2, space="PSUM") as ps_s,
            tc.tile_pool(name="ps_acc", bufs=4, space="PSUM") as ps_acc,
            tc.tile_pool(name="ps_d", bufs=2, space="PSUM") as ps_d,
            tc.tile_pool(name="dram", bufs=1, space="DRAM") as dramp,
        ):
            # Resident operands
            kT_sb = kvp.tile([128, S], f32r, name="kT_sb")
            nc.sync.dma_start(kT_sb[:], kT[:])
            v_sb = kvp.tile([128, NKT * D], f32r, name="v_sb")
            for t in range(NKT):
                nc.sync.dma_start(
                    v_sb[:, t * D : (t + 1) * D], v[t * 128 : (t + 1) * 128, :]
                )
            wT_sb = wp.tile([128, NJT * MS], f32r, name="wT_sb")
            for a in range(NJT):
                nc.sync.dma_start(
                    wT_sb[:, a * MS : (a + 1) * MS], wT[a * 128 : (a + 1) * 128, :]
                )
            ones_sb = constp.tile([128, 128], f32r, name="ones_sb")
            nc.sync.dma_start(ones_sb[:], ones_d[:])

            cc_in = dramp.tile([JS, S], f32r, name="cc_in")
            cc_out = dramp.tile([MODEL, S], f32r, name="cc_out", addr_space="Shared")

            # Attention (transposed layout, no on-device transposes)
            for c in range(NQC):
                for h in range(HPG):
                    q_sb = qtp.tile([128, QC], f32r, tag="q", name="q_sb")
                    nc.sync.dma_start(q_sb[:], qT[h, :, c * QC : (c + 1) * QC])
                    psum_o = ps_acc.tile([128, QC], f32, tag="acc", name="psum_o")
                    psum_den = ps_d.tile([1, QC], f32, tag="den", name="psum_den")
                    for t in range(NKT):
                        ps = ps_s.tile([128, QC], f32, tag="scores", name="ps")
                        nc.tensor.matmul(
                            ps[:],
                            kT_sb[:, t * 128 : (t + 1) * 128],
                            q_sb[:],
                            start=True,
                            stop=True,
                        )
                        ex = expp.tile([128, QC], f32r, tag="exp", name="ex")
                        nc.scalar.activation(ex[:], ps[:], EXP, scale=scale)
                        nc.tensor.matmul(
                            psum_den[:],
                            ones_sb[:, 0:1],
                            ex[:],
                            start=(t == 0),
                            stop=(t == NKT - 1),
                        )
                        nc.tensor.matmul(
                            psum_o[:],
                            v_sb[:, t * D : (t + 1) * D],
                            ex[:],
                            start=(t == 0),
                            stop=(t == NKT - 1),
                        )
                    recip = miscp.tile([1, QC], f32r, tag="recip", name="recip")
                    nc.vector.reciprocal(recip[:], psum_den[:])
                    ps_b = ps_s.tile([128, QC], f32, tag="scores", name="ps_b")
                    nc.tensor.matmul(
                        ps_b[:],
                        ones_sb[0:1, :],
                        recip[:],
                        start=True,
                        stop=True,
                    )
                    rb_sb = miscp.tile([128, QC], f32, tag="rb", name="rb_sb")
                    nc.scalar.activation(rb_sb[:], ps_b[:], COPY)
                    cc_sb = ccp.tile([128, QC], f32r, tag="cc", name="cc_sb")
                    nc.vector.tensor_mul(cc_sb[:], psum_o[:], rb_sb[:])
                    nc.sync.dma_start(
                        cc_in[h * 128 : (h + 1) * 128, c * QC : (c + 1) * QC],
                        cc_sb[:],
                    )

            nc.gpsimd.collective_compute(
                "AllGather",
                mybir.AluOpType.bypass,
                replica_groups=[list(range(NCORES))],
                ins=[cc_in.opt()],
                outs=[cc_out.opt()],
            )

            # Output projection: out[s, mslice] accumulated over 32 j tiles.
            for sc in range(NSC):
                psums = []
                for si in range(SC // 128):
                    psums.append(
                        ps_acc.tile([128, MS], f32, tag="acc", name="psum_p")
                    )
                for a in range(NJT):
                    lt = pip.tile([128, SC], f32r, tag="pin", name="lt")
                    nc.sync.dma_start(
                        lt[:],
                        cc_out[a * 128 : (a + 1) * 128, sc * SC : (sc + 1) * SC],
                    )
                    for si in range(SC // 128):
                        nc.tensor.matmul(
                            psums[si][:],
                            lt[:, si * 128 : (si + 1) * 128],
                            wT_sb[:, a * MS : (a + 1) * MS],
                            start=(a == 0),
                            stop=(a == NJT - 1),
                        )
                for si in range(SC // 128):
                    o_sb = outp.tile([128, MS], f32, tag="o", name="o_sb")
                    nc.scalar.activation(o_sb[:], psums[si][:], COPY)
                    nc.sync.dma_start(
                        out[(sc * 4 + si) * 128 : (sc * 4 + si + 1) * 128, :],
                        o_sb[:],
                    )
    lp.__exit__(None, None, None)
    nc.finalize()
    return nc


def _get_nc():
    if "nc" not in _CACHE:
        _CACHE["nc"] = _build_bass()
    return _CACHE["nc"]


def _make_in_maps(query, key, value, w_out):
    query = np.asarray(query, dtype=np.float32)
    key = np.asarray(key, dtype=np.float32)
    value = np.asarray(value, dtype=np.float32)
    w_out = np.asarray(w_out, dtype=np.float32)
    ones = np.ones((128, 128), dtype=np.float32)
    in_maps = []
    for g in range(NCORES):
        qTg = np.ascontiguousarray(
            query[:, g * HPG : (g + 1) * HPG, :].transpose(1, 2, 0)
        )  # [HPG, D, S]
        kTg = np.ascontiguousarray(key[:, g, :].T)  # [D, S]
        vg = np.ascontiguousarray(value[:, g, :])  # [S, D]
        wTg = np.ascontiguousarray(w_out[g * MS : (g + 1) * MS, :].T)  # [MODEL, MS]
        in_maps.append({"qT": qTg, "kT": kTg, "v": vg, "wT": wTg, "ones": ones})
    return in_maps


def run_sharded(query, key, value, w_out, trace=False, tmpdir=None):
    """Run the SPMD kernel; returns (out_full [S, MODEL], BassKernelResults)."""
    if "/opt/trn_rl_repo" not in sys.path:
        sys.path.insert(0, "/opt/trn_rl_repo")
    from concourse.bass_utils import run_bass_kernel_spmd

    nc = _get_nc()
    in_maps = _make_in_maps(query, key, value, w_out)
    res = run_bass_kernel_spmd(
        nc, in_maps, list(range(NCORES)), trace=trace, tmpdir=tmpdir
    )
    outs = [np.asarray(res.results[g]["out"]) for g in range(NCORES)]
    full = np.concatenate(outs, axis=1)  # [S, MODEL]
    return full, res


def kernel(query, key, value, mask, w_out, b_out):
    full, _ = run_sharded(query, key, value, w_out, trace=False)
    full = full + np.asarray(b_out, dtype=np.float32)[None, :]
    return full.reshape(S, H, D).astype(np.float32)



# revision 4
# speedup vs baseline: 1.1106x; 1.1106x over previous
"""Grouped-query attention + output projection on 8 trn2 NeuronCores.

Sharding: by SEQUENCE (queries).  Core i owns queries s in [i*256, (i+1)*256)
and computes ALL 32 heads for its slice; K/V (small) and w_out are replicated.
The projection input (all heads' attention outputs for the local queries) is
then entirely local -- NO collective at all, and attention outputs never
round-trip through DRAM.

Everything runs in a transposed layout so no on-device transposes are needed:

  mm1:   scoresT[k, (h2,q)] = kT_tile.T @ qT2          (contraction over D=128)
  exp:   ACT Exp over [128, 1024] PSUM (2 k-tiles), fused 1/sqrt(D) scale
  denom: ones[128,1].T @ expT -> [1, 512]              (accumulated over k)
  mm2:   outT[d, (h2,q)] = v_tile.T @ expT             (accumulated over k)
  norm:  outT * (ones x 1/denom)                       (broadcast via K=1 matmul)
  proj:  out[s_blk, m_blk] = cc_tile.T @ w_tile        (accumulated over j)

All matmul operands are bf16 (halves SBUF/DMA traffic and PE input power vs
fp32r); PSUM accumulation is fp32.  Host-side prep lays every tensor out in
the exact SBUF layout the tensor engine wants.
"""

import sys

import numpy as np

S = 2048
H = 32
G = 8
D = 128
HPG = H // G
MODEL = H * D         # 4096
NCORES = 8
SL = S // NCORES      # 256 queries per core
NP = H // 2           # 16 head-pairs per core
NKT = S // 128        # 16 k tiles
NTP = NKT // 2        # 8 k-tile pairs
NJT = MODEL // 128    # 32 j tiles (proj contraction) == heads
NMB = MODEL // 512    # 8 m blocks

_CACHE = {}


def _build_bass():
    if "/opt/trn_rl_repo" not in sys.path:
        sys.path.insert(0, "/opt/trn_rl_repo")
    import concourse.bacc as bacc
    import concourse.mybir as mybir
    import concourse.tile as tile

    f32 = mybir.dt.float32
    bf16 = mybir.dt.bfloat16
    EXP = mybir.ActivationFunctionType.Exp
    COPY = mybir.ActivationFunctionType.Copy
    scale = float(D) ** -0.5

    nc = bacc.Bacc(None, num_devices=NCORES)
    # q2[p, d, j*256+q] = query[s0+q, 2p+j, d]   (per-core)
    q2 = nc.dram_tensor("q2", [NP, D, 2 * SL], bf16, kind="ExternalInput")
    # kT[g, d, k] = key[k, g, d]                 (replicated)
    kT = nc.dram_tensor("kT", [G, D, S], bf16, kind="ExternalInput")
    # vt[g, kk, t*128+dd] = value[t*128+kk, g, dd]
    vt = nc.dram_tensor("vt", [G, 128, S], bf16, kind="ExternalInput")
    # wt[mb, a, jj, mm] = w_out[mb*512+mm, a*128+jj]
    wt = nc.dram_tensor("wt", [NMB, NJT, 128, 512], bf16, kind="ExternalInput")
    ones_d = nc.dram_tensor("ones", [128, 128], bf16, kind="ExternalInput")
    out = nc.dram_tensor("out", [SL, MODEL], f32, kind="ExternalOutput")

    lp = nc.allow_low_precision("bf16 matmul operands")
    lp.__enter__()
    with tile.TileContext(nc) as tc:
        with (
            tc.tile_pool(name="const", bufs=1) as constp,
            tc.tile_pool(name="kv", bufs=1) as kvp,
            tc.tile_pool(name="qt", bufs=3) as qtp,
            tc.tile_pool(name="expt", bufs=4) as expp,
            tc.tile_pool(name="cc", bufs=1) as ccp,
            tc.tile_pool(name="w", bufs=4) as wp,
            tc.tile_pool(name="misc", bufs=4) as miscp,
            tc.tile_pool(name="osb", bufs=3) as outp,
            tc.tile_pool(name="ps_s", bufs=2, space="PSUM") as ps_s,
            tc.tile_pool(name="ps_o", bufs=2, space="PSUM") as ps_o,
            tc.tile_pool(name="ps_d", bufs=2, space="PSUM") as ps_d,
        ):
            # Resident operands
            ones_sb = constp.tile([128, 128], bf16, name="ones_sb")
            nc.sync.dma_start(ones_sb[:], ones_d[:])
            kT_sb = kvp.tile([128, G * S], bf16, name="kT_sb")
            for g in range(G):
                nc.sync.dma_start(kT_sb[:, g * S : (g + 1) * S], kT[g])
            v_sb = kvp.tile([128, G * S], bf16, name="v_sb")
            for g in range(G):
                nc.sync.dma_start(v_sb[:, g * S : (g + 1) * S], vt[g])

            cc_tiles = []
            for p in range(NP):
                cc_tiles.append(
                    ccp.tile([128, 2 * SL], bf16, tag=f"cc{p}", name=f"cc{p}")
                )

            # ---- Attention: head-pair p covers heads (2p, 2p+1), group p//2
            for p in range(NP):
                g = p // 2
                kbase = g * S
                q_sb = qtp.tile([128, 2 * SL], bf16, tag="q", name="q_sb")
                nc.sync.dma_start(q_sb[:], q2[p])
                psum_o = ps_o.tile([128, 2 * SL], f32, tag="o", name="psum_o")
                psum_den = ps_d.tile([1, 2 * SL], f32, tag="den", name="psum_den")
                for tp in range(NTP):
                    t0 = 2 * tp
                    t1 = 2 * tp + 1
                    ps = ps_s.tile([128, 1024], f32, tag="scores", name="ps")
                    nc.tensor.matmul(
                        ps[:, 0:512],
                        kT_sb[:, kbase + t0 * 128 : kbase + t0 * 128 + 128],
                        q_sb[:],
                        start=True,
                        stop=True,
                    )
                    nc.tensor.matmul(
                        ps[:, 512:1024],
                        kT_sb[:, kbase + t1 * 128 : kbase + t1 * 128 + 128],
                        q_sb[:],
                        start=True,
                        stop=True,
                    )
                    ex = expp.tile([128, 1024], bf16, tag="exp", name="ex")
                    nc.scalar.activation(ex[:], ps[:], EXP, scale=scale)
                    for j, t in ((0, t0), (1, t1)):
                        exh = ex[:, j * 512 : (j + 1) * 512]
                        nc.tensor.matmul(
                            psum_den[:],
                            ones_sb[:, 0:1],
                            exh,
                            start=(t == 0),
                            stop=(t == NKT - 1),
                        )
                        nc.tensor.matmul(
                            psum_o[:],
                            v_sb[:, kbase + t * 128 : kbase + t * 128 + 128],
                            exh,
                            start=(t == 0),
                            stop=(t == NKT - 1),
                        )
                recip = miscp.tile([1, 2 * SL], bf16, tag="recip", name="recip")
                nc.vector.reciprocal(recip[:], psum_den[:])
                rb_ps = ps_s.tile([128, 512], f32, tag="scores", name="rb_ps")
                nc.tensor.matmul(
                    rb_ps[:], ones_sb[0:1, :], recip[:], start=True, stop=True
                )
                rb_sb = miscp.tile([128, 2 * SL], bf16, tag="rb", name="rb_sb")
                nc.scalar.activation(rb_sb[:], rb_ps[:], COPY)
                nc.vector.tensor_mul(cc_tiles[p][:], psum_o[:], rb_sb[:])

            # ---- Projection: out[si*128+s, mb*512+m] += cc[j, s] * w'[j, m]
            for mb in range(NMB):
                pp = ps_s.tile([128, 1024], f32, tag="scores", name="pp")
                for a in range(NJT):
                    w_sb = wp.tile([128, 512], bf16, tag="w", name="w_sb")
                    nc.sync.dma_start(w_sb[:], wt[mb, a])
                    for si in range(2):
                        lhs = cc_tiles[a // 2][
                            :, (a % 2) * SL + si * 128 : (a % 2) * SL + si * 128 + 128
                        ]
                        nc.tensor.matmul(
                            pp[:, si * 512 : (si + 1) * 512],
                            lhs,
                            w_sb[:],
                            start=(a == 0),
                            stop=(a == NJT - 1),
                        )
                for si in range(2):
                    o_sb = outp.tile([128, 512], f32, tag="o", name="o_sb")
                    nc.scalar.activation(o_sb[:], pp[:, si * 512 : (si + 1) * 512], COPY)
                    nc.sync.dma_start(
                        out[si * 128 : si * 128 + 128, mb * 512 : (mb + 1) * 512],
                        o_sb[:],
                    )
    lp.__exit__(None, None, None)
    nc.finalize()
    return nc


def _get_nc():
    if "nc" not in _CACHE:
        _CACHE["nc"] = _build_bass()
    return _CACHE["nc"]


def _make_in_maps(query, key, value, w_out):
    import ml_dtypes

    bf = ml_dtypes.bfloat16
    query = np.asarray(query, dtype=np.float32)
    key = np.asarray(key, dtype=np.float32)
    value = np.asarray(value, dtype=np.float32)
    w_out = np.asarray(w_out, dtype=np.float32)

    # Replicated tensors
    kT = np.ascontiguousarray(key.transpose(1, 2, 0)).astype(bf)  # [G, D, S]
    vt = np.ascontiguousarray(
        value.reshape(NKT, 128, G, D).transpose(2, 1, 0, 3).reshape(G, 128, S)
    ).astype(bf)
    wt = np.ascontiguousarray(
        w_out.reshape(NMB, 512, NJT, 128).transpose(0, 2, 3, 1)
    ).astype(bf)  # [mb, a, jj, mm]
    ones = np.ones((128, 128), dtype=bf)

    in_maps = []
    for i in range(NCORES):
        qs = query[i * SL : (i + 1) * SL]  # [SL, H, D]
        # [H, D, SL] -> [NP, 2, D, SL] -> [NP, D, 2, SL] -> [NP, D, 2*SL]
        q2 = (
            np.ascontiguousarray(
                qs.transpose(1, 2, 0)
                .reshape(NP, 2, D, SL)
                .transpose(0, 2, 1, 3)
                .reshape(NP, D, 2 * SL)
            )
        ).astype(bf)
        in_maps.append({"q2": q2, "kT": kT, "vt": vt, "wt": wt, "ones": ones})
    return in_maps


def run_sharded(query, key, value, w_out, trace=False, tmpdir=None):
    """Run the SPMD kernel; returns (out_full [S, MODEL], BassKernelResults)."""
    if "/opt/trn_rl_repo" not in sys.path:
        sys.path.insert(0, "/opt/trn_rl_repo")
    from concourse.bass_utils import run_bass_kernel_spmd

    nc = _get_nc()
    in_maps = _make_in_maps(query, key, value, w_out)
    res = run_bass_kernel_spmd(
        nc, in_maps, list(range(NCORES)), trace=trace, tmpdir=tmpdir
    )
    outs = [np.asarray(res.results[i]["out"]) for i in range(NCORES)]
    full = np.concatenate(outs, axis=0)  # [S, MODEL]
    return full, res


def kernel(query, key, value, mask, w_out, b_out):
    full, _ = run_sharded(query, key, value, w_out, trace=False)
    full = full + np.asarray(b_out, dtype=np.float32)[None, :]
    return full.reshape(S, H, D).astype(np.float32)


# revision 5
# speedup vs baseline: 1.5857x; 1.4278x over previous
"""Grouped-query attention + output projection on 8 trn2 NeuronCores.

Sharding: by SEQUENCE (queries).  Core i owns queries s in [i*256, (i+1)*256)
and computes ALL 32 heads for its slice; K/V (small) and w_out are replicated.
The projection input (all heads' attention outputs for the local queries) is
then entirely local -- NO collective at all, and attention outputs never
round-trip through DRAM.

Everything runs in a transposed layout so no on-device transposes are needed:

  mm1:   scoresT[k, (h2,q)] = kT_tile.T @ qT2          (contraction over D=128)
  exp:   ACT Exp over [128, 1024] PSUM (2 k-tiles), fused 1/sqrt(D) scale
  denom: DVE-accumulate exp tiles over k, then ONE ones[128,1].T @ acc matmul
  mm2:   outT[d, (h2,q)] = v_tile.T @ expT             (accumulated over k)
  norm:  outT * (ones x 1/denom)                       (broadcast via K=1 matmul)
  proj:  out[s_blk, m_blk] = cc_tile.T @ w_tile        (accumulated over j)

All matmul operands are bf16 (halves SBUF/DMA traffic and PE input power vs
fp32r); PSUM accumulation is fp32.  The denominator reduction rides the DVE
(bf16 2x mode) instead of burning 256 tensor-engine matmuls.  Loops are
group-major so each kT/v stationary load serves two head-pair matmuls.
"""

import sys

import numpy as np

S = 2048
H = 32
G = 8
D = 128
HPG = H // G
MODEL = H * D         # 4096
NCORES = 8
SL = S // NCORES      # 256 queries per core
NP = H // 2           # 16 head-pairs per core
NKT = S // 128        # 16 k tiles
NTP = NKT // 2        # 8 k-tile pairs
NJT = MODEL // 128    # 32 j tiles (proj contraction) == heads
NMB = MODEL // 512    # 8 m blocks

_CACHE = {}


def _build_bass():
    if "/opt/trn_rl_repo" not in sys.path:
        sys.path.insert(0, "/opt/trn_rl_repo")
    import concourse.bacc as bacc
    import concourse.mybir as mybir
    import concourse.tile as tile

    f32 = mybir.dt.float32
    bf16 = mybir.dt.bfloat16
    EXP = mybir.ActivationFunctionType.Exp
    COPY = mybir.ActivationFunctionType.Copy
    scale = float(D) ** -0.5

    nc = bacc.Bacc(None, num_devices=NCORES)
    # q2[p, d, j*256+q] = query[s0+q, 2p+j, d]   (per-core)
    q2 = nc.dram_tensor("q2", [NP, D, 2 * SL], bf16, kind="ExternalInput")
    # kT[g, d, k] = key[k, g, d]                 (replicated)
    kT = nc.dram_tensor("kT", [G, D, S], bf16, kind="ExternalInput")
    # vt[g, kk, t*128+dd] = value[t*128+kk, g, dd]
    vt = nc.dram_tensor("vt", [G, 128, S], bf16, kind="ExternalInput")
    # wt[mb, a, jj, mm] = w_out[mb*512+mm, a*128+jj]
    wt = nc.dram_tensor("wt", [NMB, NJT, 128, 512], bf16, kind="ExternalInput")
    ones_d = nc.dram_tensor("ones", [128, 128], bf16, kind="ExternalInput")
    out = nc.dram_tensor("out", [SL, MODEL], f32, kind="ExternalOutput")

    lp = nc.allow_low_precision("bf16 matmul operands")
    lp.__enter__()
    with tile.TileContext(nc) as tc:
        with (
            tc.tile_pool(name="const", bufs=1) as constp,
            tc.tile_pool(name="kv", bufs=1) as kvp,
            tc.tile_pool(name="qt", bufs=3) as qtp,
            tc.tile_pool(name="expt", bufs=6) as expp,
            tc.tile_pool(name="acc", bufs=4) as accp,
            tc.tile_pool(name="cc", bufs=1) as ccp,
            tc.tile_pool(name="w", bufs=16) as wp,
            tc.tile_pool(name="misc", bufs=4) as miscp,
            tc.tile_pool(name="osb", bufs=3) as outp,
            tc.tile_pool(name="ps_s", bufs=3, space="PSUM") as ps_s,
            tc.tile_pool(name="ps_o", bufs=2, space="PSUM") as ps_o,
        ):
            # Resident operands
            ones_sb = constp.tile([128, 128], bf16, name="ones_sb")
            nc.sync.dma_start(ones_sb[:], ones_d[:])
            kT_sb = kvp.tile([128, G * S], bf16, name="kT_sb")
            for g in range(G):
                nc.sync.dma_start(kT_sb[:, g * S : (g + 1) * S], kT[g])
            v_sb = kvp.tile([128, G * S], bf16, name="v_sb")
            for g in range(G):
                nc.sync.dma_start(v_sb[:, g * S : (g + 1) * S], vt[g])

            cc_tiles = []
            for p in range(NP):
                cc_tiles.append(
                    ccp.tile([128, 2 * SL], bf16, tag=f"cc{p}", name=f"cc{p}")
                )

            # ---- Attention, group-major: group g covers head-pairs (2g, 2g+1)
            for g in range(G):
                kbase = g * S
                q_sbs = []
                psums_o = []
                accs = []
                for hp in range(2):
                    p = 2 * g + hp
                    q_sb = qtp.tile([128, 2 * SL], bf16, tag="q", name="q_sb")
                    nc.sync.dma_start(q_sb[:], q2[p])
                    q_sbs.append(q_sb)
                    psums_o.append(
                        ps_o.tile([128, 2 * SL], f32, tag="o", name="psum_o")
                    )
                    accs.append(
                        accp.tile([128, 2 * SL], bf16, tag="acc", name="acc")
                    )
                exs = [None, None]
                for tp in range(NTP):
                    t0 = 2 * tp
                    t1 = 2 * tp + 1
                    pss = []
                    for hp in range(2):
                        pss.append(
                            ps_s.tile([128, 1024], f32, tag="scores", name="ps")
                        )
                    # two mm1 per stationary kT tile
                    for j, t in ((0, t0), (1, t1)):
                        for hp in range(2):
                            nc.tensor.matmul(
                                pss[hp][:, j * 512 : (j + 1) * 512],
                                kT_sb[:, kbase + t * 128 : kbase + t * 128 + 128],
                                q_sbs[hp][:],
                                start=True,
                                stop=True,
                            )
                    for hp in range(2):
                        ex = expp.tile([128, 1024], bf16, tag="exp", name="ex")
                        nc.scalar.activation(ex[:], pss[hp][:], EXP, scale=scale)
                        exs[hp] = ex
                    # two mm2 per stationary v tile
                    for j, t in ((0, t0), (1, t1)):
                        for hp in range(2):
                            nc.tensor.matmul(
                                psums_o[hp][:],
                                v_sb[:, kbase + t * 128 : kbase + t * 128 + 128],
                                exs[hp][:, j * 512 : (j + 1) * 512],
                                start=(t == 0),
                                stop=(t == NKT - 1),
                            )
                    # DVE: accumulate denominator partials
                    for hp in range(2):
                        if tp == 0:
                            nc.vector.tensor_add(
                                accs[hp][:], exs[hp][:, 0:512], exs[hp][:, 512:1024]
                            )
                        else:
                            nc.vector.tensor_add(
                                accs[hp][:], accs[hp][:], exs[hp][:, 0:512]
                            )
                            nc.vector.tensor_add(
                                accs[hp][:], accs[hp][:], exs[hp][:, 512:1024]
                            )
                # normalize: denom = ones.T @ acc; recip; broadcast; cc = o * rb
                for hp in range(2):
                    p = 2 * g + hp
                    ps_den = ps_s.tile([1, 2 * SL], f32, tag="scores", name="ps_den")
                    nc.tensor.matmul(
                        ps_den[:], ones_sb[:, 0:1], accs[hp][:], start=True, stop=True
                    )
                    recip = miscp.tile([1, 2 * SL], bf16, tag="recip", name="recip")
                    nc.vector.reciprocal(recip[:], ps_den[:])
                    rb_ps = ps_s.tile([128, 512], f32, tag="scores", name="rb_ps")
                    nc.tensor.matmul(
                        rb_ps[:], ones_sb[0:1, :], recip[:], start=True, stop=True
                    )
                    rb_sb = miscp.tile([128, 2 * SL], bf16, tag="rb", name="rb_sb")
                    nc.scalar.activation(rb_sb[:], rb_ps[:], COPY)
                    nc.vector.tensor_mul(cc_tiles[p][:], psums_o[hp][:], rb_sb[:])

            # ---- Projection: out[si*128+s, mb*512+m] += cc[j, s] * w'[j, m]
            for mb in range(NMB):
                pp = ps_s.tile([128, 1024], f32, tag="scores", name="pp")
                for a in range(NJT):
                    w_sb = wp.tile([128, 512], bf16, tag="w", name="w_sb")
                    nc.sync.dma_start(w_sb[:], wt[mb, a])
                    for si in range(2):
                        lhs = cc_tiles[a // 2][
                            :, (a % 2) * SL + si * 128 : (a % 2) * SL + si * 128 + 128
                        ]
                        nc.tensor.matmul(
                            pp[:, si * 512 : (si + 1) * 512],
                            lhs,
                            w_sb[:],
                            start=(a == 0),
                            stop=(a == NJT - 1),
                        )
                for si in range(2):
                    o_sb = outp.tile([128, 512], f32, tag="o", name="o_sb")
                    nc.scalar.activation(o_sb[:], pp[:, si * 512 : (si + 1) * 512], COPY)
                    nc.sync.dma_start(
                        out[si * 128 : si * 128 + 128, mb * 512 : (mb + 1) * 512],
                        o_sb[:],
                    )
    lp.__exit__(None, None, None)
    nc.finalize()
    return nc


def _get_nc():
    if "nc" not in _CACHE:
        _CACHE["nc"] = _build_bass()
    return _CACHE["nc"]


def _make_in_maps(query, key, value, w_out):
    import ml_dtypes

    bf = ml_dtypes.bfloat16
    query = np.asarray(query, dtype=np.float32)
    key = np.asarray(key, dtype=np.float32)
    value = np.asarray(value, dtype=np.float32)
    w_out = np.asarray(w_out, dtype=np.float32)

    # Replicated tensors
    kT = np.ascontiguousarray(key.transpose(1, 2, 0)).astype(bf)  # [G, D, S]
    vt = np.ascontiguousarray(
        value.reshape(NKT, 128, G, D).transpose(2, 1, 0, 3).reshape(G, 128, S)
    ).astype(bf)
    wt = np.ascontiguousarray(
        w_out.reshape(NMB, 512, NJT, 128).transpose(0, 2, 3, 1)
    ).astype(bf)  # [mb, a, jj, mm]
    ones = np.ones((128, 128), dtype=bf)

    in_maps = []
    for i in range(NCORES):
        qs = query[i * SL : (i + 1) * SL]  # [SL, H, D]
        # [H, D, SL] -> [NP, 2, D, SL] -> [NP, D, 2, SL] -> [NP, D, 2*SL]
        q2 = (
            np.ascontiguousarray(
                qs.transpose(1, 2, 0)
                .reshape(NP, 2, D, SL)
                .transpose(0, 2, 1, 3)
                .reshape(NP, D, 2 * SL)
            )
        ).astype(bf)
        in_maps.append({"q2": q2, "kT": kT, "vt": vt, "wt": wt, "ones": ones})
    return in_maps


def run_sharded(query, key, value, w_out, trace=False, tmpdir=None):
    """Run the SPMD kernel; returns (out_full [S, MODEL], BassKernelResults)."""
    if "/opt/trn_rl_repo" not in sys.path:
        sys.path.insert(0, "/opt/trn_rl_repo")
    from concourse.bass_utils import run_bass_kernel_spmd

    nc = _get_nc()
    in_maps = _make_in_maps(query, key, value, w_out)
    res = run_bass_kernel_spmd(
        nc, in_maps, list(range(NCORES)), trace=trace, tmpdir=tmpdir
    )
    outs = [np.asarray(res.results[i]["out"]) for i in range(NCORES)]
    full = np.concatenate(outs, axis=0)  # [S, MODEL]
    return full, res


def kernel(query, key, value, mask, w_out, b_out):
    full, _ = run_sharded(query, key, value, w_out, trace=False)
    full = full + np.asarray(b_out, dtype=np.float32)[None, :]
    return full.reshape(S, H, D).astype(np.float32)


# revision 13
# speedup vs baseline: 1.8618x; 1.1741x over previous
"""Grouped-query attention + output projection on 8 trn2 NeuronCores.

Sharding: by SEQUENCE (queries).  Core i owns queries s in [i*256, (i+1)*256)
and computes ALL 32 heads for its slice; K/V (small) and w_out are replicated.
The projection input (all heads' attention outputs for the local queries) is
then entirely local -- NO collective at all, and attention outputs never
round-trip through DRAM.

Everything runs in a transposed layout so no on-device transposes are needed:

  mm1:   scoresT[k, (h2,q)] = kT_tile.T @ qT2          (contraction over D=128)
  exp:   ACT Exp over [128, 1024] PSUM (2 k-tiles), fused 1/sqrt(D) scale
  denom: DVE-accumulate exp tiles over k, then ONE ones[128,1].T @ acc matmul
  mm2:   outT[d, (h2,q)] = v_tile.T @ expT             (accumulated over k)
  norm:  outT * (ones x 1/denom)                       (broadcast via K=1 matmul)
  proj:  out[s_blk, m_blk] = cc_tile.T @ w_tile        (accumulated over j)

All matmul operands are bf16 (halves SBUF/DMA traffic and PE input power vs
fp32r); PSUM accumulation is fp32.  The denominator reduction rides the DVE
(bf16 2x mode) instead of burning 256 tensor-engine matmuls.  Loops are
group-major so each kT/v stationary load serves two head-pair matmuls.
"""

import sys

import numpy as np

S = 2048
H = 32
G = 8
D = 128
HPG = H // G
MODEL = H * D         # 4096
NCORES = 8
SL = S // NCORES      # 256 queries per core
NP = H // 2           # 16 head-pairs per core
NKT = S // 128        # 16 k tiles
NTP = NKT // 2        # 8 k-tile pairs
NJT = MODEL // 128    # 32 j tiles (proj contraction) == heads
NMB = MODEL // 512    # 8 m blocks

_CACHE = {}


def _build_bass():
    if "/opt/trn_rl_repo" not in sys.path:
        sys.path.insert(0, "/opt/trn_rl_repo")
    import concourse.bacc as bacc
    import concourse.mybir as mybir
    import concourse.tile as tile

    f32 = mybir.dt.float32
    bf16 = mybir.dt.bfloat16
    EXP = mybir.ActivationFunctionType.Exp
    COPY = mybir.ActivationFunctionType.Copy
    scale = float(D) ** -0.5

    nc = bacc.Bacc(None, num_devices=NCORES)
    # q2[p, d, j*256+q] = query[s0+q, 2p+j, d]   (per-core)
    q2 = nc.dram_tensor("q2", [NP, D, 2 * SL], bf16, kind="ExternalInput")
    # kT[g, d, k] = key[k, g, d]                 (replicated)
    kT = nc.dram_tensor("kT", [G, D, S], bf16, kind="ExternalInput")
    # vt[g, kk, t*128+dd] = value[t*128+kk, g, dd]
    vt = nc.dram_tensor("vt", [G, 128, S], bf16, kind="ExternalInput")
    # wt[mb, jj, a*512+mm] = w_out[mb*512+mm, a*128+jj]  (one slab per m-block)
    wt = nc.dram_tensor("wt", [NMB, 128, NJT * 512], bf16, kind="ExternalInput")
    ones_d = nc.dram_tensor("ones", [128, 128], bf16, kind="ExternalInput")
    out = nc.dram_tensor("out", [SL, MODEL], f32, kind="ExternalOutput")

    lp = nc.allow_low_precision("bf16 matmul operands")
    lp.__enter__()
    with tile.TileContext(nc) as tc:
        with (
            tc.tile_pool(name="const", bufs=1) as constp,
            tc.tile_pool(name="kv", bufs=1) as kvp,
            tc.tile_pool(name="qt", bufs=3) as qtp,
            tc.tile_pool(name="expt", bufs=6) as expp,
            tc.tile_pool(name="acc", bufs=6) as accp,
            tc.tile_pool(name="cc", bufs=1) as ccp,
            tc.tile_pool(name="w", bufs=2) as wp,
            tc.tile_pool(name="misc", bufs=4) as miscp,
            tc.tile_pool(name="osb", bufs=3) as outp,
            tc.tile_pool(name="ps_s", bufs=3, space="PSUM") as ps_s,
            tc.tile_pool(name="ps_o", bufs=2, space="PSUM") as ps_o,
        ):
            # Resident operands
            ones_sb = constp.tile([128, 128], bf16, name="ones_sb")
            nc.sync.dma_start(ones_sb[:], ones_d[:])
            kT_sb = kvp.tile([128, G * S], bf16, name="kT_sb")
            for g in range(G):
                nc.sync.dma_start(kT_sb[:, g * S : (g + 1) * S], kT[g])
            v_sb = kvp.tile([128, G * S], bf16, name="v_sb")
            for g in range(G):
                nc.sync.dma_start(v_sb[:, g * S : (g + 1) * S], vt[g])

            cc_tiles = []
            for p in range(NP):
                cc_tiles.append(
                    ccp.tile([128, 2 * SL], bf16, tag=f"cc{p}", name=f"cc{p}")
                )

            # ---- Attention, group-major: group g covers head-pairs (2g, 2g+1)
            for g in range(G):
                kbase = g * S
                q_sbs = []
                psums_o = []
                accs = []
                for hp in range(2):
                    p = 2 * g + hp
                    q_sb = qtp.tile([128, 2 * SL], bf16, tag="q", name="q_sb")
                    nc.sync.dma_start(q_sb[:], q2[p])
                    q_sbs.append(q_sb)
                    psums_o.append(
                        ps_o.tile([128, 2 * SL], f32, tag="o", name="psum_o")
                    )
                    accs.append(None)
                exs = [None, None]
                for tp in range(NTP):
                    t0 = 2 * tp
                    t1 = 2 * tp + 1
                    pss = []
                    for hp in range(2):
                        pss.append(
                            ps_s.tile([128, 1024], f32, tag="scores", name="ps")
                        )
                    # two mm1 per stationary kT tile
                    for j, t in ((0, t0), (1, t1)):
                        for hp in range(2):
                            nc.tensor.matmul(
                                pss[hp][:, j * 512 : (j + 1) * 512],
                                kT_sb[:, kbase + t * 128 : kbase + t * 128 + 128],
                                q_sbs[hp][:],
                                start=True,
                                stop=True,
                            )
                    for hp in range(2):
                        ex = expp.tile([128, 1024], bf16, tag="exp", name="ex")
                        nc.scalar.activation(ex[:], pss[hp][:], EXP, scale=scale)
                        exs[hp] = ex
                    # two mm2 per stationary v tile
                    for j, t in ((0, t0), (1, t1)):
                        for hp in range(2):
                            nc.tensor.matmul(
                                psums_o[hp][:],
                                v_sb[:, kbase + t * 128 : kbase + t * 128 + 128],
                                exs[hp][:, j * 512 : (j + 1) * 512],
                                start=(t == 0),
                                stop=(t == NKT - 1),
                            )
                    # DVE: accumulate denominator partials (ping-pong, never
                    # in-place, so the bf16 2x perf mode can engage)
                    for hp in range(2):
                        a0 = accp.tile([128, 2 * SL], bf16, tag="acc", name="acc")
                        if tp == 0:
                            nc.vector.tensor_add(
                                a0[:], exs[hp][:, 0:512], exs[hp][:, 512:1024]
                            )
                        else:
                            a1 = accp.tile(
                                [128, 2 * SL], bf16, tag="acc", name="acc"
                            )
                            nc.vector.tensor_add(
                                a1[:], accs[hp][:], exs[hp][:, 0:512]
                            )
                            nc.vector.tensor_add(
                                a0[:], a1[:], exs[hp][:, 512:1024]
                            )
                        accs[hp] = a0
                # normalize: denom = ones.T @ acc; recip; broadcast; cc = o * rb
                for hp in range(2):
                    p = 2 * g + hp
                    ps_den = ps_s.tile([1, 2 * SL], f32, tag="scores", name="ps_den")
                    nc.tensor.matmul(
                        ps_den[:], ones_sb[:, 0:1], accs[hp][:], start=True, stop=True
                    )
                    recip = miscp.tile([1, 2 * SL], bf16, tag="recip", name="recip")
                    nc.vector.reciprocal(recip[:], ps_den[:])
                    rb_ps = ps_s.tile([128, 512], f32, tag="scores", name="rb_ps")
                    nc.tensor.matmul(
                        rb_ps[:], ones_sb[0:1, :], recip[:], start=True, stop=True
                    )
                    rb_sb = miscp.tile([128, 2 * SL], bf16, tag="rb", name="rb_sb")
                    nc.scalar.activation(rb_sb[:], rb_ps[:], COPY)
                    nc.vector.tensor_mul(cc_tiles[p][:], psums_o[hp][:], rb_sb[:])

            # ---- Projection: out[si*128+s, mb*512+m] += cc[j, s] * w'[j, m]
            for mb in range(NMB):
                w_sb = wp.tile([128, NJT * 512], bf16, tag="w", name="w_sb")
                nc.sync.dma_start(w_sb[:], wt[mb])
                pp = ps_s.tile([128, 1024], f32, tag="scores", name="pp")
                for a in range(NJT):
                    for si in range(2):
                        lhs = cc_tiles[a // 2][
                            :, (a % 2) * SL + si * 128 : (a % 2) * SL + si * 128 + 128
                        ]
                        nc.tensor.matmul(
                            pp[:, si * 512 : (si + 1) * 512],
                            lhs,
                            w_sb[:, a * 512 : (a + 1) * 512],
                            start=(a == 0),
                            stop=(a == NJT - 1),
                        )
                for si in range(2):
                    o_sb = outp.tile([128, 512], f32, tag="o", name="o_sb")
                    nc.scalar.activation(o_sb[:], pp[:, si * 512 : (si + 1) * 512], COPY)
                    nc.sync.dma_start(
                        out[si * 128 : si * 128 + 128, mb * 512 : (mb + 1) * 512],
                        o_sb[:],
                    )
    lp.__exit__(None, None, None)
    nc.finalize()
    return nc


def _get_nc():
    if "nc" not in _CACHE:
        _CACHE["nc"] = _build_bass()
    return _CACHE["nc"]


def _make_in_maps(query, key, value, w_out):
    import ml_dtypes

    bf = ml_dtypes.bfloat16
    query = np.asarray(query, dtype=np.float32)
    key = np.asarray(key, dtype=np.float32)
    value = np.asarray(value, dtype=np.float32)
    w_out = np.asarray(w_out, dtype=np.float32)

    # Replicated tensors
    kT = np.ascontiguousarray(key.transpose(1, 2, 0)).astype(bf)  # [G, D, S]
    vt = np.ascontiguousarray(
        value.reshape(NKT, 128, G, D).transpose(2, 1, 0, 3).reshape(G, 128, S)
    ).astype(bf)
    wt = np.ascontiguousarray(
        w_out.reshape(NMB, 512, NJT, 128)
        .transpose(0, 3, 2, 1)
        .reshape(NMB, 128, NJT * 512)
    ).astype(bf)  # [mb, jj, a*512+mm]
    ones = np.ones((128, 128), dtype=bf)

    in_maps = []
    for i in range(NCORES):
        qs = query[i * SL : (i + 1) * SL]  # [SL, H, D]
        # [H, D, SL] -> [NP, 2, D, SL] -> [NP, D, 2, SL] -> [NP, D, 2*SL]
        q2 = (
            np.ascontiguousarray(
                qs.transpose(1, 2, 0)
                .reshape(NP, 2, D, SL)
                .transpose(0, 2, 1, 3)
                .reshape(NP, D, 2 * SL)
            )
        ).astype(bf)
        in_maps.append({"q2": q2, "kT": kT, "vt": vt, "wt": wt, "ones": ones})
    return in_maps


def run_sharded(query, key, value, w_out, trace=False, tmpdir=None):
    """Run the SPMD kernel; returns (out_full [S, MODEL], BassKernelResults)."""
    if "/opt/trn_rl_repo" not in sys.path:
        sys.path.insert(0, "/opt/trn_rl_repo")
    from concourse.bass_utils import run_bass_kernel_spmd

    nc = _get_nc()
    in_maps = _make_in_maps(query, key, value, w_out)
    res = run_bass_kernel_spmd(
        nc, in_maps, list(range(NCORES)), trace=trace, tmpdir=tmpdir
    )
    outs = [np.asarray(res.results[i]["out"]) for i in range(NCORES)]
    full = np.concatenate(outs, axis=0)  # [S, MODEL]
    return full, res


def kernel(query, key, value, mask, w_out, b_out):
    full, _ = run_sharded(query, key, value, w_out, trace=False)
    full = full + np.asarray(b_out, dtype=np.float32)[None, :]
    return full.reshape(S, H, D).astype(np.float32)
